# revision 18
# baseline (speedup 1.0000x reference)
"""CorrelationFusion Trainium2 kernel.

Per-clip math (T=8 frames, G=4 groups, 3x3 correlation window):
  corr[g, tt*9+ij, p] = sum_cp x[tt, g*64+cp, p] * xpad[tt+1, g*64+cp, p+d(ij)]
  wx[g, o*8+t, p]     = sum_i conv_w[g, o*8+t, i]*corr[g, i, p] + conv_b[g, o*8+t]
  out[o, g*64+cp, p]  = sum_t wx'[g, o*8+t, p] * x[t, cp*4+g, p]
  (wx' = wx + 1 on the t==o rows -- the residual folded into the conv bias)

Mapping:
  - per-pixel products on VectorE in bf16 (2x mode), channels on partitions
  - partition reductions (over cp / over t) via TensorE ones-matmuls into PSUM
  - the 1x1 grouped conv is a block-diagonal matmul over the 63 corr rows
  - wx rows replicated into the (cp16, t8)-interleaved layout via SBUF DMA
  - data-parallel over the 8 clips: one clip per NeuronCore
"""

import numpy as np
import ml_dtypes

T = 8
TO = 8
G = 4
CPG = 64
C = 256
H = 56
W = 56
PIX = H * W
NCORES = 8
PH = 58   # padded tile rows
PW = 60   # padded tile cols (extra pad for 4B alignment of bf16 rows)
NCH = 7   # pixel chunks per image
CHW = 8   # rows per chunk
CHN = CHW * W  # 448 pixels per chunk

_CACHE = {}


def _build_module():
    import concourse.bass as bass
    import concourse.bacc as bacc
    import concourse.mybir as mybir
    import concourse.tile as tile

    fp32 = mybir.dt.float32
    bf16 = mybir.dt.bfloat16

    nc = bacc.Bacc(name="corrfusion")
    xin = nc.dram_tensor("xin", [T, C, H, W], fp32, kind="ExternalInput")
    wf2 = nc.dram_tensor("wf2", [128, 2, 128], fp32, kind="ExternalInput")
    bmat = nc.dram_tensor("bmat", [128, 251], bf16, kind="ExternalInput")
    tones = nc.dram_tensor("tones", [128, 4, 128], bf16, kind="ExternalInput")
    bvec = nc.dram_tensor("bvec", [128, 2], fp32, kind="ExternalInput")
    out = nc.dram_tensor("out", [TO, C, H, W], fp32, kind="ExternalOutput")

    xin_flat = xin.rearrange("t c h w -> t c (h w)")          # [8, 256, 3136]
    xin_base = xin[:, :, :, :]                                 # base AP for manual APs
    out_r = out.rearrange("o (g cpc k) h w -> o g cpc k (h w)", g=4, cpc=4, k=16)

    with tile.TileContext(nc) as tc:
        with tc.tile_pool(name="consts", bufs=1) as consts, \
             tc.tile_pool(name="corrbuf", bufs=1) as corrbuf, \
             tc.tile_pool(name="stage", bufs=2) as stage:

            wf_sb = consts.tile([128, 2, 128], fp32)
            nc.sync.dma_start(out=wf_sb, in_=wf2[:, :, :])
            bm_sb = consts.tile([128, 251], bf16)
            nc.sync.dma_start(out=bm_sb, in_=bmat[:, :])
            to_sb = consts.tile([128, 4, 128], bf16)
            nc.sync.dma_start(out=to_sb, in_=tones[:, :, :])
            bv_sb = consts.tile([128, 2], fp32)
            nc.sync.dma_start(out=bv_sb, in_=bvec[:, :])

            # corr rows 0..62: even group of the pair; 63..125: odd group;
            # 126: ones (bias row); 127: zeros
            corr_sb = [
                corrbuf.tile([128, PIX], fp32, tag=f"corr{i}", name=f"corr{i}")
                for i in range(2)
            ]
            wx_sb = [
                corrbuf.tile([128, PIX], bf16, tag=f"wx{i}", name=f"wx{i}")
                for i in range(2)
            ]
            # rows 96..127 pre-zeroed so the unused k-rows 126/127 of the conv
            # matmul read finite zeros (drains later overwrite 96..125)
            for i in range(2):
                nc.vector.memset(corr_sb[i][96:128, :], 0.0)

            # ---------------- correlation phase ----------------
            with tc.tile_pool(name="frames", bufs=1) as frames, \
                 tc.tile_pool(name="prods", bufs=3) as prods, \
                 tc.tile_pool(name="cpsum", bufs=1, space="PSUM") as cpsum:
                for ct in range(2):
                    cps = [
                        cpsum.tile([126, CHN], fp32, tag=f"cp{c}", name=f"cps{ct}_{c}")
                        for c in range(NCH)
                    ]
                    ptile = {}
                    stile = {}

                    def load_frame(t, ct=ct, ptile=ptile, stile=stile):
                        stg = stage.tile([128, PIX], fp32, tag="fstage", name="stg")
                        nc.sync.dma_start(out=stg, in_=xin_flat[t, ct * 128:(ct + 1) * 128, :])
                        stg3 = stg.rearrange("p (h w) -> p h w", h=H)
                        P = frames.tile([128, PH, PW], bf16, tag=f"P{t % 4}", name=f"P{ct}_{t}")
                        S = frames.tile([128, PH, PW], bf16, tag=f"S{t % 4}", name=f"S{ct}_{t}")
                        # cast fp32 -> bf16 into padded centers (ScalarE)
                        nc.scalar.copy(P[:, 1:57, 2:58], stg3)
                        nc.scalar.copy(S[:, 1:57, 1:57], stg3)
                        # edge replication pads (GpSimd), cols then rows
                        nc.gpsimd.tensor_copy(P[:, 1:57, 1:2], P[:, 1:57, 2:3])
                        nc.gpsimd.tensor_copy(P[:, 1:57, 58:59], P[:, 1:57, 57:58])
                        nc.gpsimd.tensor_copy(P[:, 0:1, 1:59], P[:, 1:2, 1:59])
                        nc.gpsimd.tensor_copy(P[:, 57:58, 1:59], P[:, 56:57, 1:59])
                        nc.gpsimd.tensor_copy(S[:, 1:57, 0:1], S[:, 1:57, 1:2])
                        nc.gpsimd.tensor_copy(S[:, 1:57, 57:58], S[:, 1:57, 56:57])
                        nc.gpsimd.tensor_copy(S[:, 0:1, 0:58], S[:, 1:2, 0:58])
                        nc.gpsimd.tensor_copy(S[:, 57:58, 0:58], S[:, 56:57, 0:58])
                        ptile[t] = P
                        stile[t] = S

                    load_frame(0)
                    load_frame(1)
                    for tt in range(T - 1):
                        if tt + 2 < T:
                            load_frame(tt + 2)
                        a = ptile[tt][:, 1:57, 2:58]
                        for ij in range(9):
                            di = ij // 3 - 1
                            dj = ij % 3 - 1
                            r = tt * 9 + ij
                            if dj == 0:
                                b = ptile[tt + 1][:, 1 + di:57 + di, 2:58]
                            elif dj == 1:
                                b = stile[tt + 1][:, 1 + di:57 + di, 2:58]
                            else:
                                b = stile[tt + 1][:, 1 + di:57 + di, 0:56]
                            pr = prods.tile([128, PIX], bf16, tag="prod", name="pr")
                            pr3 = pr.rearrange("p (h w) -> p h w", h=H)
                            nc.vector.tensor_mul(pr3, a, b)
                            lhsT = bm_sb[:, 125 - r:251 - r]
                            for c in range(NCH):
                                nc.tensor.matmul(
                                    cps[c],
                                    lhsT,
                                    pr[:, c * CHN:(c + 1) * CHN],
                                    start=(r == 0),
                                    stop=(r == 62),
                                )
                    for c in range(NCH):
                        nc.scalar.copy(
                            corr_sb[ct][0:126, c * CHN:(c + 1) * CHN],
                            cps[c],
                        )

            # ---------------- conv + weighted-frame-sum phase ----------------
            with tc.tile_pool(name="xt", bufs=2) as xtp, \
                 tc.tile_pool(name="wrep", bufs=2) as wrepp, \
                 tc.tile_pool(name="pr2", bufs=1) as pr2p, \
                 tc.tile_pool(name="xob", bufs=2) as xobp, \
                 tc.tile_pool(name="wpsum", bufs=2, space="PSUM") as wps, \
                 tc.tile_pool(name="xpsum", bufs=2, space="PSUM") as xps:

                # grouped 1x1 conv (+bias +residual), both group-pairs
                for gp in range(2):
                    for c in range(NCH):
                        wpp = wps.tile([128, CHN], fp32, tag="wp", name="wpp")
                        nc.tensor.matmul(
                            wpp,
                            wf_sb[:, gp, :],
                            corr_sb[gp][:, c * CHN:(c + 1) * CHN],
                            start=True,
                            stop=True,
                        )
                        nc.scalar.activation(
                            wx_sb[gp][:, c * CHN:(c + 1) * CHN],
                            wpp,
                            mybir.ActivationFunctionType.Identity,
                            bias=bv_sb[:, gp:gp + 1],
                            scale=1.0,
                        )

                for g in range(G):
                    xts = []
                    for cpc in range(4):
                        stg = stage.tile([128, PIX], fp32, tag="fstage", name="stgx")
                        # partition = (cpk, t): channel c = cpc*64 + cpk*4 + g
                        src = bass.AP(
                            tensor=xin_base.tensor,
                            offset=(cpc * 64 + g) * PIX,
                            ap=[[4 * PIX, 16], [C * PIX, T], [1, PIX]],
                        )
                        nc.sync.dma_start(out=stg, in_=src)
                        xt = xtp.tile([128, PIX], bf16, tag=f"xt{cpc}", name=f"xt{g}_{cpc}")
                        nc.scalar.copy(xt, stg)
                        xts.append(xt)
                    for o in range(TO):
                        rowbase = (g % 2) * 64 + o * 8
                        wrep = wrepp.tile([128, PIX], bf16, tag="wrep", name="wrep")
                        for cpk in range(16):
                            nc.sync.dma_start(
                                out=wrep[cpk * 8:(cpk + 1) * 8, :],
                                in_=wx_sb[g // 2][rowbase:rowbase + 8, :],
                            )
                        pr2s = []
                        for cpc in range(4):
                            pr2 = pr2p.tile(
                                [128, PIX], bf16, tag=f"pr2{cpc}", name=f"pr2_{cpc}"
                            )
                            nc.vector.tensor_mul(pr2, xts[cpc], wrep)
                            pr2s.append(pr2)
                        xout = xobp.tile([128, PIX], fp32, tag="xout", name="xout")
                        for c in range(NCH):
                            xop = xps.tile([128, CHN], fp32, tag="xo", name="xop")
                            for cpc in range(4):
                                nc.tensor.matmul(
                                    xop,
                                    to_sb[:, cpc, :],
                                    pr2s[cpc][:, c * CHN:(c + 1) * CHN],
                                    start=(cpc == 0),
                                    stop=(cpc == 3),
                                )
                            nc.scalar.copy(xout[:, c * CHN:(c + 1) * CHN], xop)
                        for cpc in range(4):
                            nc.sync.dma_start(
                                out=out_r[o, g, cpc, :, :],
                                in_=xout[32 * cpc:32 * cpc + 16, :],
                            )
    nc.compile()
    return nc


def _get_module():
    if "nc" not in _CACHE:
        _CACHE["nc"] = _build_module()
    return _CACHE["nc"]


def _consts(conv_w, conv_b):
    conv_w = np.asarray(conv_w, np.float32)
    conv_b = np.asarray(conv_b, np.float32)
    # block-diagonal fused conv weights per group-pair:
    #   wf2[k, gp, m]; m = (g%2)*64 + o*8 + t; k rows (g%2)*63..+63 hold
    #   conv_w[g, o*8+t, :].  Bias (+1.0 residual when t==o) applied at the
    #   PSUM drain as a per-partition activation bias (bvec).
    wf2 = np.zeros((128, 2, 128), np.float32)
    bvec = np.zeros((128, 2), np.float32)
    for gp in range(2):
        for gh in range(2):
            g = gp * 2 + gh
            half = gh * 63
            for o in range(TO):
                for t in range(T):
                    m = gh * 64 + o * 8 + t
                    wf2[half:half + 63, gp, m] = conv_w[g, o * 8 + t]
                    bvec[m, gp] = conv_b[g, o * 8 + t] + (1.0 if t == o else 0.0)

    bm = np.zeros((128, 251), np.float32)
    bm[0:64, 125] = 1.0
    bm[64:128, 188] = 1.0

    # t-reduce ones: tones[p=(cpk,t), cpc, m] = 1 iff m == 32*cpc + cpk
    to = np.zeros((128, 4, 128), np.float32)
    for cpc in range(4):
        for cpk in range(16):
            to[cpk * 8:(cpk + 1) * 8, cpc, 32 * cpc + cpk] = 1.0

    return (
        wf2,
        bm.astype(ml_dtypes.bfloat16),
        to.astype(ml_dtypes.bfloat16),
        bvec,
    )


def kernel(x, conv_w, conv_b):
    from concourse.bass_utils import run_bass_kernel_spmd

    nc = _get_module()
    wf, bm, to, bv = _consts(conv_w, conv_b)
    x = np.ascontiguousarray(np.asarray(x, np.float32))
    x8 = x.reshape(NCORES, T, C, H, W)
    in_maps = [
        {
            "xin": np.ascontiguousarray(x8[i]),
            "wf2": wf,
            "bmat": bm,
            "tones": to,
            "bvec": bv,
        }
        for i in range(NCORES)
    ]
    res = run_bass_kernel_spmd(nc, in_maps, core_ids=list(range(NCORES)))
    outs = [r["out"] for r in res.results]
    return np.concatenate(outs, axis=0).astype(np.float32)


# revision 21
# speedup vs baseline: 38.7815x; 38.7815x over previous
"""CorrelationFusion Trainium2 kernel.

Per-clip math (T=8 frames, G=4 groups, 3x3 correlation window):
  corr[g, tt*9+ij, p] = sum_cp x[tt, g*64+cp, p] * xpad[tt+1, g*64+cp, p+d(ij)]
  wx[g, o*8+t, p]     = sum_i conv_w[g, o*8+t, i]*corr[g, i, p] + conv_b[g, o*8+t]
  out[o, g*64+cp, p]  = sum_t wx'[g, o*8+t, p] * x[t, cp*4+g, p]
  (wx' = wx + 1 on the t==o rows -- the residual folded into the conv bias)

Mapping:
  - per-pixel products on VectorE in bf16 (2x mode), channels on partitions
  - partition reductions (over cp / over t) via TensorE ones-matmuls into PSUM
  - the 1x1 grouped conv is a block-diagonal matmul over the 63 corr rows
  - wx rows replicated into the (cp16, t8)-interleaved layout via SBUF DMA
  - data-parallel over the 8 clips: one clip per NeuronCore
"""

import numpy as np
import ml_dtypes

T = 8
TO = 8
G = 4
CPG = 64
C = 256
H = 56
W = 56
PIX = H * W
NCORES = 8
PH = 58   # padded tile rows
PW = 60   # padded tile cols (extra pad for 4B alignment of bf16 rows)
NCH = 7   # pixel chunks per image
CHW = 8   # rows per chunk
CHN = CHW * W  # 448 pixels per chunk

_CACHE = {}


def _build_module(loop_k=1):
    import contextlib

    import concourse.bass as bass
    import concourse.bacc as bacc
    import concourse.mybir as mybir
    import concourse.tile as tile

    fp32 = mybir.dt.float32
    bf16 = mybir.dt.bfloat16

    nc = bacc.Bacc(name="corrfusion")
    xin = nc.dram_tensor("xin", [T, C, H, W], fp32, kind="ExternalInput")
    wf2 = nc.dram_tensor("wf2", [128, 2, 128], fp32, kind="ExternalInput")
    bmat = nc.dram_tensor("bmat", [128, 251], bf16, kind="ExternalInput")
    tones = nc.dram_tensor("tones", [128, 4, 128], bf16, kind="ExternalInput")
    bvec = nc.dram_tensor("bvec", [128, 2], fp32, kind="ExternalInput")
    out = nc.dram_tensor("out", [TO, C, H, W], fp32, kind="ExternalOutput")

    xin_flat = xin.rearrange("t c h w -> t c (h w)")          # [8, 256, 3136]
    xin_base = xin[:, :, :, :]                                 # base AP for manual APs
    out_r = out.rearrange("o (g cpc k) h w -> o g cpc k (h w)", g=4, cpc=4, k=16)

    with tile.TileContext(nc) as tc:
        with tc.tile_pool(name="consts", bufs=1) as consts, \
             tc.tile_pool(name="corrbuf", bufs=1) as corrbuf, \
             tc.tile_pool(name="stage", bufs=2) as stage:

            wf_sb = consts.tile([128, 2, 128], fp32)
            nc.sync.dma_start(out=wf_sb, in_=wf2[:, :, :])
            bm_sb = consts.tile([128, 251], bf16)
            nc.sync.dma_start(out=bm_sb, in_=bmat[:, :])
            to_sb = consts.tile([128, 4, 128], bf16)
            nc.sync.dma_start(out=to_sb, in_=tones[:, :, :])
            bv_sb = consts.tile([128, 2], fp32)
            nc.sync.dma_start(out=bv_sb, in_=bvec[:, :])

            # corr rows 0..62: even group of the pair; 63..125: odd group;
            # 126: ones (bias row); 127: zeros
            corr_sb = [
                corrbuf.tile([128, PIX], fp32, tag=f"corr{i}", name=f"corr{i}")
                for i in range(2)
            ]
            wx_sb = [
                corrbuf.tile([128, PIX], bf16, tag=f"wx{i}", name=f"wx{i}")
                for i in range(2)
            ]
            # benchmark mode: run the whole per-clip body loop_k times
            loop_cm = (
                tc.For_i(0, loop_k, 1) if loop_k > 1 else contextlib.nullcontext()
            )
            with loop_cm:
                _build_body(nc, tc, tile, bass, mybir, fp32, bf16, xin_flat,
                            xin_base, out_r, wf_sb, bm_sb, to_sb, bv_sb, corr_sb,
                            wx_sb, stage)
    nc.compile()
    return nc


def _build_body(nc, tc, tile, bass, mybir, fp32, bf16, xin_flat, xin_base,
                out_r, wf_sb, bm_sb, to_sb, bv_sb, corr_sb, wx_sb, stage):
    G = 4
    if True:
        if True:
            # rows 96..127 pre-zeroed so the unused k-rows 126/127 of the conv
            # matmul read finite zeros (drains later overwrite 96..125)
            for i in range(2):
                nc.vector.memset(corr_sb[i][96:128, :], 0.0)

            # ---------------- correlation phase ----------------
            with tc.tile_pool(name="frames", bufs=1) as frames, \
                 tc.tile_pool(name="prods", bufs=3) as prods, \
                 tc.tile_pool(name="cpsum", bufs=1, space="PSUM") as cpsum:
                for ct in range(2):
                    cps = [
                        cpsum.tile([126, CHN], fp32, tag=f"cp{c}", name=f"cps{ct}_{c}")
                        for c in range(NCH)
                    ]
                    ptile = {}
                    stile = {}

                    def load_frame(t, ct=ct, ptile=ptile, stile=stile):
                        stg = stage.tile([128, PIX], fp32, tag="fstage", name="stg")
                        nc.sync.dma_start(out=stg, in_=xin_flat[t, ct * 128:(ct + 1) * 128, :])
                        stg3 = stg.rearrange("p (h w) -> p h w", h=H)
                        P = frames.tile([128, PH, PW], bf16, tag=f"P{t % 4}", name=f"P{ct}_{t}")
                        S = frames.tile([128, PH, PW], bf16, tag=f"S{t % 4}", name=f"S{ct}_{t}")
                        # cast fp32 -> bf16 into padded centers (ScalarE)
                        nc.scalar.copy(P[:, 1:57, 2:58], stg3)
                        nc.scalar.copy(S[:, 1:57, 1:57], stg3)
                        # edge replication pads (GpSimd), cols then rows
                        nc.gpsimd.tensor_copy(P[:, 1:57, 1:2], P[:, 1:57, 2:3])
                        nc.gpsimd.tensor_copy(P[:, 1:57, 58:59], P[:, 1:57, 57:58])
                        nc.gpsimd.tensor_copy(P[:, 0:1, 1:59], P[:, 1:2, 1:59])
                        nc.gpsimd.tensor_copy(P[:, 57:58, 1:59], P[:, 56:57, 1:59])
                        nc.gpsimd.tensor_copy(S[:, 1:57, 0:1], S[:, 1:57, 1:2])
                        nc.gpsimd.tensor_copy(S[:, 1:57, 57:58], S[:, 1:57, 56:57])
                        nc.gpsimd.tensor_copy(S[:, 0:1, 0:58], S[:, 1:2, 0:58])
                        nc.gpsimd.tensor_copy(S[:, 57:58, 0:58], S[:, 56:57, 0:58])
                        ptile[t] = P
                        stile[t] = S

                    load_frame(0)
                    load_frame(1)
                    for tt in range(T - 1):
                        if tt + 2 < T:
                            load_frame(tt + 2)
                        a = ptile[tt][:, 1:57, 2:58]
                        for ij in range(9):
                            di = ij // 3 - 1
                            dj = ij % 3 - 1
                            r = tt * 9 + ij
                            if dj == 0:
                                b = ptile[tt + 1][:, 1 + di:57 + di, 2:58]
                            elif dj == 1:
                                b = stile[tt + 1][:, 1 + di:57 + di, 2:58]
                            else:
                                b = stile[tt + 1][:, 1 + di:57 + di, 0:56]
                            pr = prods.tile([128, PIX], bf16, tag="prod", name="pr")
                            pr3 = pr.rearrange("p (h w) -> p h w", h=H)
                            nc.vector.tensor_mul(pr3, a, b)
                            lhsT = bm_sb[:, 125 - r:251 - r]
                            for c in range(NCH):
                                nc.tensor.matmul(
                                    cps[c],
                                    lhsT,
                                    pr[:, c * CHN:(c + 1) * CHN],
                                    start=(r == 0),
                                    stop=(r == 62),
                                )
                    for c in range(NCH):
                        nc.scalar.copy(
                            corr_sb[ct][0:126, c * CHN:(c + 1) * CHN],
                            cps[c],
                        )

            # ---------------- conv + weighted-frame-sum phase ----------------
            with tc.tile_pool(name="xt", bufs=2) as xtp, \
                 tc.tile_pool(name="wrep", bufs=2) as wrepp, \
                 tc.tile_pool(name="pr2", bufs=1) as pr2p, \
                 tc.tile_pool(name="xob", bufs=2) as xobp, \
                 tc.tile_pool(name="wpsum", bufs=2, space="PSUM") as wps, \
                 tc.tile_pool(name="xpsum", bufs=2, space="PSUM") as xps:

                # grouped 1x1 conv (+bias +residual), both group-pairs
                for gp in range(2):
                    for c in range(NCH):
                        wpp = wps.tile([128, CHN], fp32, tag="wp", name="wpp")
                        nc.tensor.matmul(
                            wpp,
                            wf_sb[:, gp, :],
                            corr_sb[gp][:, c * CHN:(c + 1) * CHN],
                            start=True,
                            stop=True,
                        )
                        nc.scalar.activation(
                            wx_sb[gp][:, c * CHN:(c + 1) * CHN],
                            wpp,
                            mybir.ActivationFunctionType.Identity,
                            bias=bv_sb[:, gp:gp + 1],
                            scale=1.0,
                        )

                for g in range(G):
                    xts = []
                    for cpc in range(4):
                        stg = stage.tile([128, PIX], fp32, tag="fstage", name="stgx")
                        # partition = (cpk, t): channel c = cpc*64 + cpk*4 + g
                        src = bass.AP(
                            tensor=xin_base.tensor,
                            offset=(cpc * 64 + g) * PIX,
                            ap=[[4 * PIX, 16], [C * PIX, T], [1, PIX]],
                        )
                        nc.sync.dma_start(out=stg, in_=src)
                        xt = xtp.tile([128, PIX], bf16, tag=f"xt{cpc}", name=f"xt{g}_{cpc}")
                        nc.scalar.copy(xt, stg)
                        xts.append(xt)
                    for o in range(TO):
                        rowbase = (g % 2) * 64 + o * 8
                        wrep = wrepp.tile([128, PIX], bf16, tag="wrep", name="wrep")
                        for cpk in range(16):
                            nc.sync.dma_start(
                                out=wrep[cpk * 8:(cpk + 1) * 8, :],
                                in_=wx_sb[g // 2][rowbase:rowbase + 8, :],
                            )
                        pr2s = []
                        for cpc in range(4):
                            pr2 = pr2p.tile(
                                [128, PIX], bf16, tag=f"pr2{cpc}", name=f"pr2_{cpc}"
                            )
                            nc.vector.tensor_mul(pr2, xts[cpc], wrep)
                            pr2s.append(pr2)
                        xout = xobp.tile([128, PIX], fp32, tag="xout", name="xout")
                        for c in range(NCH):
                            xop = xps.tile([128, CHN], fp32, tag="xo", name="xop")
                            for cpc in range(4):
                                nc.tensor.matmul(
                                    xop,
                                    to_sb[:, cpc, :],
                                    pr2s[cpc][:, c * CHN:(c + 1) * CHN],
                                    start=(cpc == 0),
                                    stop=(cpc == 3),
                                )
                            nc.scalar.copy(xout[:, c * CHN:(c + 1) * CHN], xop)
                        for cpc in range(4):
                            nc.sync.dma_start(
                                out=out_r[o, g, cpc, :, :],
                                in_=xout[32 * cpc:32 * cpc + 16, :],
                            )


def _get_module(loop_k=1):
    key = f"nc{loop_k}"
    if key not in _CACHE:
        _CACHE[key] = _build_module(loop_k)
    return _CACHE[key]


def _consts(conv_w, conv_b):
    conv_w = np.asarray(conv_w, np.float32)
    conv_b = np.asarray(conv_b, np.float32)
    # block-diagonal fused conv weights per group-pair:
    #   wf2[k, gp, m]; m = (g%2)*64 + o*8 + t; k rows (g%2)*63..+63 hold
    #   conv_w[g, o*8+t, :].  Bias (+1.0 residual when t==o) applied at the
    #   PSUM drain as a per-partition activation bias (bvec).
    wf2 = np.zeros((128, 2, 128), np.float32)
    bvec = np.zeros((128, 2), np.float32)
    for gp in range(2):
        for gh in range(2):
            g = gp * 2 + gh
            half = gh * 63
            for o in range(TO):
                for t in range(T):
                    m = gh * 64 + o * 8 + t
                    wf2[half:half + 63, gp, m] = conv_w[g, o * 8 + t]
                    bvec[m, gp] = conv_b[g, o * 8 + t] + (1.0 if t == o else 0.0)

    bm = np.zeros((128, 251), np.float32)
    bm[0:64, 125] = 1.0
    bm[64:128, 188] = 1.0

    # t-reduce ones: tones[p=(cpk,t), cpc, m] = 1 iff m == 32*cpc + cpk
    to = np.zeros((128, 4, 128), np.float32)
    for cpc in range(4):
        for cpk in range(16):
            to[cpk * 8:(cpk + 1) * 8, cpc, 32 * cpc + cpk] = 1.0

    return (
        wf2,
        bm.astype(ml_dtypes.bfloat16),
        to.astype(ml_dtypes.bfloat16),
        bvec,
    )


def kernel(x, conv_w, conv_b):
    from concourse.bass_utils import run_bass_kernel_spmd

    nc = _get_module()
    wf, bm, to, bv = _consts(conv_w, conv_b)
    x = np.ascontiguousarray(np.asarray(x, np.float32))
    x8 = x.reshape(NCORES, T, C, H, W)
    in_maps = [
        {
            "xin": np.ascontiguousarray(x8[i]),
            "wf2": wf,
            "bmat": bm,
            "tones": to,
            "bvec": bv,
        }
        for i in range(NCORES)
    ]
    res = run_bass_kernel_spmd(nc, in_maps, core_ids=list(range(NCORES)))
    outs = [r["out"] for r in res.results]
    return np.concatenate(outs, axis=0).astype(np.float32)
